# revision 1
# baseline (speedup 1.0000x reference)
"""Distributed multi-head attention kernel for 8 TRN2 NeuronCores — v2.

Problem: B=2, N=2048, C=768, H=12 heads of dim 64.
Sharding: core i owns batch i//4 and global heads {r, r+4, r+8}, r=i%4
(strided so each AllToAll slot j delivers a contiguous 256-channel block).

Per head j (self-paired scores): qT/kT blocks hold the head's 64 dims in
rows 0:64 AND a duplicate in rows 64:128, so two score matmuls (K=64) run
concurrently in the PE array at tile positions (0,0)/(64,0) — 2x throughput.
Softmax exp is split between ACT (exact, bf16 out) and DVE (Schraudolph
int16-bitcast exp, ~1.4% rms) to double softmax throughput.  PV is K=128
M=65 with a trailing ones column producing the denominator row.
Normalization: reciprocal_approx_fast + DRAM-round-trip partition broadcast.
A tiny dummy AllToAll at kernel start absorbs the collective-init barrier
and inter-core dispatch skew; per-head AllToAlls then run at ~7us and
overlap the next head's compute.  The output projection accumulates over
6 jc-blocks jc-outer so its first chunks overlap the last AllToAll.
"""

import numpy as np
import ml_dtypes

import concourse.bass as bass
import concourse.mybir as mybir
import concourse.tile as tile
from concourse import bacc
from concourse.bass_utils import run_bass_kernel_spmd

B, N, C, H, HD = 2, 2048, 768, 12, 64
SCALE = HD ** -0.5          # 0.125
P = 128
CB = C // P                 # 6 channel blocks
KB = N // P                 # 16 key blocks
QCH = 512
NQC = N // QCH              # 4
HPC = 3                     # heads per core
NCORES = 8
VW = HPC * (HD + 1)         # 195
RQ = N // NCORES            # 256
PVLAG = 10

f32 = mybir.dt.float32
bf16 = mybir.dt.bfloat16
i16 = mybir.dt.int16
Exp = mybir.ActivationFunctionType.Exp
Identity = mybir.ActivationFunctionType.Identity

# Schraudolph exp constants (bf16 bit space), scale folded in
LOG2E = 1.4426950408889634
SEXP_A = 128 * LOG2E * SCALE
SEXP_B = 127 * 128 - 4.7


def _body(nc, tc, xT, wqkT, wvT, woT, bo_d, out_d, dbg=None):
    with (
        tc.tile_pool(name="const", bufs=1) as constp,
        tc.tile_pool(name="big", bufs=1) as bigp,
        tc.tile_pool(name="esp", bufs=PVLAG + 3) as esp,
        tc.tile_pool(name="smallp", bufs=2) as smallp,
        tc.tile_pool(name="normp", bufs=1) as normp,
        tc.tile_pool(name="outp", bufs=2) as outp,
        tc.tile_pool(name="psS", bufs=2, space="PSUM") as psS,
        tc.tile_pool(name="psC", bufs=1, space="PSUM") as psC,
        tc.tile_pool(name="dram", bufs=1, space="DRAM") as dramp,
    ):
        # ---- dummy A2A first: absorbs CC-init barrier + core dispatch skew
        dummy_sb = constp.tile([8, 64], bf16, name="dummy_sb")
        nc.vector.memset(dummy_sb[:, :], 0.0)
        send_d = dramp.tile([NCORES, 1, 64], bf16, name="send_d")
        recv_d = dramp.tile([NCORES, 1, 64], bf16, name="recv_d")
        nc.sync.dma_start(send_d[:, 0, :], dummy_sb[:, :])
        nc.gpsimd.collective_compute(
            "AllToAll", mybir.AluOpType.bypass,
            replica_groups=[list(range(NCORES))],
            ins=[send_d.opt()], outs=[recv_d.opt()])

        # ---- load inputs ----
        xT_sb = [bigp.tile([P, N], bf16, name=f"xT_sb_{cb}") for cb in range(CB)]
        wqkT_sb = bigp.tile([P, CB * 384], bf16, name="wqkT_sb")
        wvT_sb = bigp.tile([P, CB * 192], bf16, name="wvT_sb")
        woT_sb = bigp.tile([P, CB * C], bf16, name="woT_sb")
        bo_sb = bigp.tile([P, CB], f32, name="bo_sb")
        ones_sb = constp.tile([P, 1], f32, name="ones_sb")
        nc.vector.memset(ones_sb[:, :], 1.0)
        warm_sb = constp.tile([P, 1], f32, name="warm_sb")
        nc.scalar.activation(warm_sb[0:1, :], ones_sb[0:1, 0:1], Exp, scale=SCALE)
        # weights on the ACT DMA ring, x on the sync ring: parallel transfers
        for cb in range(CB):
            nc.scalar.dma_start(wqkT_sb[:, cb * 384:(cb + 1) * 384], wqkT[cb * P:(cb + 1) * P, :])
        for cb in range(CB):
            nc.sync.dma_start(xT_sb[cb][:, :], xT[cb * P:(cb + 1) * P, :])
            nc.scalar.dma_start(wvT_sb[:, cb * 192:(cb + 1) * 192], wvT[cb * P:(cb + 1) * P, :])
        for cb in range(CB):
            nc.scalar.dma_start(woT_sb[:, cb * C:(cb + 1) * C], woT[cb * P:(cb + 1) * P, :])
            nc.scalar.dma_start(bo_sb[:, cb:cb + 1], bo_d[cb * P:(cb + 1) * P, :])

        # ---- PE warmup during loads: un-throttle HAM before projections ----
        wtile = constp.tile([P, 640], bf16, name="wtile")
        nc.vector.memset(wtile[:, :], 0.001)
        for w in range(24):
            wps = psS.tile([P, 2 * QCH], f32, name=f"wps_{w}", tag="psS")
            nc.tensor.matmul(wps[:, 0:QCH], lhsT=wtile[:, 0:128],
                             rhs=wtile[:, 128:640], start=True, stop=True)

        # ---- Q/K projections: per head j, psum = [q_j (rows 0:64) | k_j (64:128)]
        qT_sb = bigp.tile([P, HPC * N], bf16, name="qT_sb")
        kT_sb = bigp.tile([P, HPC * N], bf16, name="kT_sb")
        for j in range(HPC):
            for qn in range(NQC):
                ps = psS.tile([P, 2 * QCH], f32, name=f"pj_{j}_{qn}", tag="psS")
                for cb in range(CB):
                    nc.tensor.matmul(
                        ps[:, 0:QCH],
                        lhsT=wqkT_sb[:, cb * 384 + j * 128: cb * 384 + (j + 1) * 128],
                        rhs=xT_sb[cb][:, qn * QCH: (qn + 1) * QCH],
                        start=(cb == 0), stop=(cb == CB - 1),
                    )
                cols = slice(j * N + qn * QCH, j * N + (qn + 1) * QCH)
                nc.vector.tensor_copy(qT_sb[0:64, cols], ps[0:64, 0:QCH])
                nc.scalar.copy(kT_sb[64:128, cols], ps[64:128, 0:QCH])
            # duplicate head dims into the other partition half (row-tile pairing)
            blk = slice(j * N, (j + 1) * N)
            nc.sync.dma_start(qT_sb[64:128, blk], qT_sb[0:64, blk])
            nc.sync.dma_start(kT_sb[0:64, blk], kT_sb[64:128, blk])

        # ---- V projection: nb 11..15 here; nb 0..10 interleaved into head 0
        v_sb = bigp.tile([P, KB * VW], bf16, name="v_sb")

        def vproj(nb, pool_tag):
            if pool_tag == "psC":
                ps = psC.tile([P, 192], f32, name=f"vps_{nb}", tag="psC")
            else:
                ps = psS.tile([P, 2 * QCH], f32, name=f"vps_{nb}", tag="psS")
            for cb in range(CB):
                nc.tensor.matmul(
                    ps[:, 0:192],
                    lhsT=xT_sb[cb][:, nb * P:(nb + 1) * P],
                    rhs=wvT_sb[:, cb * 192:(cb + 1) * 192],
                    start=(cb == 0), stop=(cb == CB - 1),
                )
            vv = v_sb[:, nb * VW:(nb + 1) * VW].rearrange("p (h w) -> p h w", h=HPC)
            pp = ps[:, 0:192].rearrange("p (h w) -> p h w", h=HPC)
            nc.vector.tensor_copy(vv[:, :, 0:64], pp[:, :, :])
            nc.vector.memset(vv[:, :, 64:65], 1.0)

        for nb in range(11, KB):
            vproj(nb, "psS")

        # ---- attention: software-pipelined across heads ----
        # per head j: 16 kb-steps of paired scores; own PV covers kb2 0..4 at
        # kb 11..15; kb2 5..15 drains qc-major during the NEXT head's kb 0..7,
        # with per-qc norm chains pipelined behind the drain.
        ctxT_sb = bigp.tile([64, HPC * N], bf16, name="ctxT_sb")
        ctxTf_sb = [bigp.tile([P, 2 * 2 * RQ], bf16, name=f"ctxTf_sb_{j}")
                    for j in range(HPC)]
        OWN = 5

        def cden_qc(jp, cps, qc):
            cden = smallp.tile([65, QCH], f32, name=f"cden_{jp}_{qc}", tag="rec")
            nc.scalar.copy(cden[64:65, :], cps[64:65, qc * QCH:(qc + 1) * QCH])
            rtmp = dramp.tile([1, QCH], f32, name=f"rtmp_{jp}_{qc}")
            nc.sync.dma_start(rtmp[:, :], cden[64:65, :])
            nc.sync.dma_start(db_t[jp % 2][0:64, qc * QCH:(qc + 1) * QCH],
                              rtmp[0:1, :].partition_broadcast(64))

        def partb_qc(jp, cps, qc, send_h):
            cs = slice(qc * QCH, (qc + 1) * QCH)
            rb = rb_t[jp % 2]
            nc.vector.reciprocal_approx_fast(out=rb[0:64, cs], in_=db_t[jp % 2][0:64, cs])
            nc.vector.tensor_mul(
                ctxT_sb[0:64, jp * N + qc * QCH: jp * N + (qc + 1) * QCH],
                cps[0:64, cs], rb[0:64, cs])
            for d in (2 * qc, 2 * qc + 1):
                nc.sync.dma_start(send_h[d, :, :],
                                  ctxT_sb[:, jp * N + d * RQ: jp * N + (d + 1) * RQ])

        def a2a(jp, send_h):
            recv_h = dramp.tile([NCORES, 64, RQ], bf16, name=f"recv_{jp}")
            nc.gpsimd.collective_compute(
                "AllToAll", mybir.AluOpType.bypass,
                replica_groups=[list(range(NCORES))],
                ins=[send_h.opt()], outs=[recv_h.opt()])
            return recv_h

        def scatter(jp, recv_h):
            # issued on the (otherwise idle) GPSIMD queue: these wait on the
            # collective, and on the shared sync queue that head-of-line wait
            # blocks every later DMA in the kernel
            for s in range(NCORES):
                u, ro = divmod((s % 4) * 64, P)
                co = (s // 4) * RQ
                nc.gpsimd.dma_start(
                    ctxTf_sb[jp][ro:ro + 64, u * 2 * RQ + co: u * 2 * RQ + co + RQ],
                    recv_h[s, :, :])

        def pv_step(jp, cps, kb2, es_list, start=False, stop=False):
            for qc in range(NQC):
                t = es_list[kb2][qc // 2]
                nc.tensor.matmul(
                    cps[0:65, qc * QCH:(qc + 1) * QCH],
                    lhsT=v_sb[:, kb2 * VW + jp * 65: kb2 * VW + (jp + 1) * 65],
                    rhs=t[:, (qc % 2) * QCH:(qc % 2 + 1) * QCH],
                    start=start, stop=stop)

        def drain_chunk_qc(jp, cps, c, es_list):
            # tail only: qc-major so each qc's denominator completes early
            qc, half = divmod(c, 2)
            kb2s = range(OWN + 6 * half, OWN + 6) if half == 0 else range(OWN + 6, KB)
            for kb2 in kb2s:
                t = es_list[kb2][qc // 2]
                nc.tensor.matmul(
                    cps[0:65, qc * QCH:(qc + 1) * QCH],
                    lhsT=v_sb[:, kb2 * VW + jp * 65: kb2 * VW + (jp + 1) * 65],
                    rhs=t[:, (qc % 2) * QCH:(qc % 2 + 1) * QCH],
                    start=False, stop=(kb2 == KB - 1))

        db_t = [normp.tile([64, N], f32, name=f"db_{i}") for i in range(2)]
        rb_t = [normp.tile([64, N], f32, name=f"rb_{i}") for i in range(2)]
        es_store = []
        recvs = []
        sends = []
        prev_cps = None
        cps = None
        for j in range(HPC):
            prev_cps = cps
            cps = None
            es_list = []
            es_store.append(es_list)
            for kb in range(KB):
                kcol = slice(j * N + kb * P, j * N + (kb + 1) * P)
                es_pair = []
                for qp in range(2):
                    sps = psS.tile([P, 2 * QCH], f32, name=f"sps_{j}_{kb}_{qp}", tag="psS")
                    for half in range(2):
                        qc = qp * 2 + half
                        ro = slice(half * 64, (half + 1) * 64)
                        nc.tensor.matmul(
                            sps[:, half * QCH:(half + 1) * QCH],
                            lhsT=kT_sb[ro, kcol],
                            rhs=qT_sb[ro, j * N + qc * QCH: j * N + (qc + 1) * QCH],
                            start=True, stop=True,
                        )
                    if qp == 0:
                        es = esp.tile([P, 2 * QCH], bf16, name=f"esA_{j}_{kb}", tag="esA")
                        nc.scalar.activation(es, sps, Exp, scale=SCALE)
                        es_pair.append(es)
                    else:
                        es16 = esp.tile([P, 2 * QCH], i16, name=f"esD_{j}_{kb}", tag="esD")
                        nc.vector.tensor_scalar(
                            out=es16[:, :], in0=sps[:, :],
                            scalar1=SEXP_A, scalar2=SEXP_B,
                            op0=mybir.AluOpType.mult, op1=mybir.AluOpType.add)
                        es_pair.append(es16.bitcast(bf16))
                es_list.append(es_pair)

                if j == 0:
                    if kb <= 10:
                        vproj(kb, "psC")
                else:
                    # drain prev head kb2-major, 2 per kb (kb2 5..15), so es
                    # tiles release promptly and the pool never deadlocks
                    if kb <= 5:
                        for sub in range(2):
                            kb2 = OWN + 2 * kb + sub
                            if kb2 < KB:
                                pv_step(j - 1, prev_cps, kb2, es_store[j - 1],
                                        stop=(kb2 == KB - 1))
                    if kb == 6:
                        for qc in range(NQC):
                            cden_qc(j - 1, prev_cps, qc)
                    if kb in (8, 9):
                        for qc in (0, 1) if kb == 8 else (2, 3):
                            partb_qc(j - 1, prev_cps, qc, sends[j - 1])
                        if kb == 9:
                            recvs.append(a2a(j - 1, sends[j - 1]))
                            scatter(j - 1, recvs[j - 1])
                if kb == 0:
                    sh = dramp.tile([NCORES, 64, RQ], bf16, name=f"send_{j}")
                    sends.append(sh)
                if kb >= KB - OWN:
                    kb2 = kb - (KB - OWN)
                    if kb2 == 0:
                        cps = psC.tile([P, N], f32, name=f"cps_{j}", tag="psC")
                    for qc in range(NQC):
                        t = es_list[kb2][qc // 2]
                        nc.tensor.matmul(
                            cps[0:65, qc * QCH:(qc + 1) * QCH],
                            lhsT=v_sb[:, kb2 * VW + j * 65: kb2 * VW + (j + 1) * 65],
                            rhs=t[:, (qc % 2) * QCH:(qc % 2 + 1) * QCH],
                            start=(kb2 == 0), stop=False)

        # ---- tail: drain head 2 + per-qc norm + A2A2, overlapped with outproj
        ops_s = [psS.tile([P, 2 * QCH], f32, name=f"ops_s{i}", tag="psS") for i in range(2)]
        ops_ap = [ops_s[0][:, 0:QCH], ops_s[0][:, QCH:2 * QCH],
                  ops_s[1][:, 0:QCH], ops_s[1][:, QCH:2 * QCH]]

        def outproj(jcs, cbos):
            for jc in jcs:
                for cbo in cbos:
                    nc.tensor.matmul(
                        ops_ap[cbo],
                        lhsT=woT_sb[:, jc * C + cbo * P: jc * C + (cbo + 1) * P],
                        rhs=ctxTf_sb[jc // 2][:, (jc % 2) * 2 * RQ:(jc % 2 + 1) * 2 * RQ],
                        start=(jc == 0), stop=(jc == CB - 1),
                    )

        for c in range(8):
            drain_chunk_qc(2, cps, c, es_store[2])
            if c in (1, 3, 5, 7):
                cden_qc(2, cps, (c - 1) // 2)
            if c == 6:
                partb_qc(2, cps, 0, sends[2])
                partb_qc(2, cps, 1, sends[2])
        partb_qc(2, cps, 2, sends[2])
        partb_qc(2, cps, 3, sends[2])
        recvs.append(a2a(2, sends[2]))
        scatter(2, recvs[2])
        outproj(range(4), range(4))
        ops_c = psC.tile([P, N], f32, name="ops_c", tag="psC")
        ops_ap.append(ops_c[:, 0:QCH])
        ops_ap.append(ops_c[:, QCH:2 * QCH])
        outproj(range(4), (4, 5))

        if dbg is not None:
            nc.sync.dma_start(dbg["qT"][:, :], qT_sb[:, :])
            nc.sync.dma_start(dbg["kT"][:, :], kT_sb[:, :])
            nc.sync.dma_start(dbg["v"][:, :], v_sb[:, :])
            nc.sync.dma_start(dbg["ctxT"][:, :], ctxT_sb[:, :])
            for j in range(HPC):
                nc.sync.dma_start(dbg["ctxTf"][:, j * 1024:(j + 1) * 1024], ctxTf_sb[j][:, :])

        # final contraction chunk (jc 4,5 <- A2A2) cbo-major, with bias+store
        # pipelined behind the next cbo's matmuls
        for cbo in range(CB):
            outproj((4, 5), [cbo])
            osb = outp.tile([P, 2 * RQ], f32, name=f"osb_{cbo}", tag="osb")
            nc.scalar.activation(osb, ops_ap[cbo], Identity, bias=bo_sb[:, cbo:cbo + 1])
            nc.sync.dma_start(out_d[cbo * P:(cbo + 1) * P, :], osb)


def build(debug_outs=False):
    nc = bacc.Bacc("TRN2", target_bir_lowering=False, debug=False, num_devices=NCORES)
    xT = nc.dram_tensor("xT", [C, N], bf16, kind="ExternalInput").ap()
    wqkT = nc.dram_tensor("wqkT", [C, HPC * 128], bf16, kind="ExternalInput").ap()
    wvT = nc.dram_tensor("wvT", [C, HPC * HD], bf16, kind="ExternalInput").ap()
    woT = nc.dram_tensor("woT", [C, C], bf16, kind="ExternalInput").ap()
    bo_d = nc.dram_tensor("bo", [C, 1], f32, kind="ExternalInput").ap()
    out_d = nc.dram_tensor("out", [C, 2 * RQ], f32, kind="ExternalOutput").ap()
    dbg = None
    if debug_outs:
        dbg = {
            "qT": nc.dram_tensor("dbg_qT", [P, HPC * N], bf16, kind="ExternalOutput").ap(),
            "kT": nc.dram_tensor("dbg_kT", [P, HPC * N], bf16, kind="ExternalOutput").ap(),
            "v": nc.dram_tensor("dbg_v", [P, KB * VW], bf16, kind="ExternalOutput").ap(),
            "ctxT": nc.dram_tensor("dbg_ctxT", [64, HPC * N], bf16, kind="ExternalOutput").ap(),
            "ctxTf": nc.dram_tensor("dbg_ctxTf", [P, CB * 2 * RQ], bf16, kind="ExternalOutput").ap(),
        }
    with tile.TileContext(nc) as tc:
        _body(nc, tc, xT, wqkT, wvT, woT, bo_d, out_d, dbg)
    nc.compile()
    return nc


_NC = None


def _get_nc():
    global _NC
    if _NC is None:
        _NC = build()
    return _NC


def make_in_maps(x, Wq, Wk, Wv, Wo, bo):
    x = np.asarray(x, np.float32)
    woT = np.ascontiguousarray(np.asarray(Wo, np.float32).T).astype(ml_dtypes.bfloat16)
    bo_col = np.ascontiguousarray(np.asarray(bo, np.float32).reshape(C, 1))
    Wq = np.asarray(Wq, np.float32)
    Wk = np.asarray(Wk, np.float32)
    Wv = np.asarray(Wv, np.float32)
    in_maps = []
    for i in range(NCORES):
        b = i // 4
        r = i % 4
        heads = [r, r + 4, r + 8]
        # wqk columns per head block j: [q_hj (64) | k_hj (64)]
        blocks = []
        for h in heads:
            hs = slice(h * HD, (h + 1) * HD)
            blocks.append(Wq[hs])
            blocks.append(Wk[hs])
        wqk = np.concatenate(blocks, axis=0).T          # [768, 384]
        wv_rows = np.concatenate([Wv[h * HD:(h + 1) * HD] for h in heads], axis=0)
        in_maps.append({
            "xT": np.ascontiguousarray(x[b].T).astype(ml_dtypes.bfloat16),
            "wqkT": np.ascontiguousarray(wqk).astype(ml_dtypes.bfloat16),
            "wvT": np.ascontiguousarray(wv_rows.T).astype(ml_dtypes.bfloat16),
            "woT": woT,
            "bo": bo_col,
        })
    return in_maps


def unshard(results):
    out = np.empty((B, N, C), np.float32)
    for i, r in enumerate(results):
        o = r["out"]  # [768, 512]: cols 0-255 batch 0, 256-511 batch 1
        out[0, i * RQ:(i + 1) * RQ, :] = o[:, :RQ].T
        out[1, i * RQ:(i + 1) * RQ, :] = o[:, RQ:].T
    return out


def kernel(x, Wq, Wk, Wv, Wo, bo):
    nc = _get_nc()
    in_maps = make_in_maps(x, Wq, Wk, Wv, Wo, bo)
    res = run_bass_kernel_spmd(nc, in_maps, core_ids=list(range(NCORES)))
    return unshard(res.results)



# revision 2
# speedup vs baseline: 1.0849x; 1.0849x over previous
"""Distributed multi-head attention kernel for 8 TRN2 NeuronCores — v3.

Problem: B=2, N=2048, C=768, H=12 heads of dim 64.
Sharding: core i owns batch i//4 and global heads {r, r+4, r+8}, r=i%4.

v3 restructures the schedule query-half-major: phase A computes all three
heads' attention for queries 0:1023, phase B for 1024:2047, each followed
by an AllToAll that redistributes context so core s ends up owning query
stripes {s*128..} of both halves and both batches.  This fires the last
collective much earlier in every core's program (the old per-head scheme
sent head 2's context at ~95% of the program), so the inter-core dispatch
skew is paid while useful work remains, and the post-collective tail is
just half the output projection.

Per (head, phase): 16 kb steps of paired K=64 score matmuls (head dims
duplicated in partition halves 0:64/64:128 so the two 512-query matmuls
run concurrently in the PE array), exp split between ACT (exact) and DVE
(Schraudolph int16-bitcast) per half, PV (K=128, M=65 with a trailing
ones column producing the denominator row) interleaved behind exp with a
small lag.  Normalization: reciprocal_approx_fast + DRAM-round-trip
partition broadcast.  A dummy AllToAll at kernel start absorbs the
collective-init barrier.
"""

import numpy as np
import ml_dtypes

import concourse.bass as bass
import concourse.mybir as mybir
import concourse.tile as tile
from concourse import bacc
from concourse.bass_utils import run_bass_kernel_spmd

B, N, C, H, HD = 2, 2048, 768, 12, 64
SCALE = HD ** -0.5          # 0.125
P = 128
CB = C // P                 # 6 channel blocks
KB = N // P                 # 16 key blocks
QCH = 512
HPC = 3                     # heads per core
NCORES = 8
VW = HPC * (HD + 1)         # 195
QP = 1024                   # queries per phase
RQ = QP // NCORES           # 128 queries per dest per phase
PVLAG = 3

f32 = mybir.dt.float32
bf16 = mybir.dt.bfloat16
i16 = mybir.dt.int16
Exp = mybir.ActivationFunctionType.Exp
Identity = mybir.ActivationFunctionType.Identity

# Schraudolph exp constants (bf16 bit space), scale folded in
LOG2E = 1.4426950408889634
SEXP_A = 128 * LOG2E * SCALE
SEXP_B = 127 * 128 - 4.7


def _body(nc, tc, xT, wqkT, wvT, woT, bo_d, out_d, dbg=None):
    with (
        tc.tile_pool(name="const", bufs=1) as constp,
        tc.tile_pool(name="big", bufs=1) as bigp,
        tc.tile_pool(name="esp", bufs=2 * PVLAG + 6) as esp,
        tc.tile_pool(name="smallp", bufs=2) as smallp,
        tc.tile_pool(name="normp", bufs=1) as normp,
        tc.tile_pool(name="outp", bufs=2) as outp,
        tc.tile_pool(name="psS", bufs=4, space="PSUM") as psS,
        tc.tile_pool(name="psC", bufs=2, space="PSUM") as psC,
        tc.tile_pool(name="dram", bufs=1, space="DRAM") as dramp,
    ):
        # ---- dummy A2A first: absorbs CC-init barrier + core dispatch skew
        dummy_sb = constp.tile([8, 64], bf16, name="dummy_sb")
        nc.vector.memset(dummy_sb[:, :], 0.0)
        send_d = dramp.tile([NCORES, 1, 64], bf16, name="send_d")
        recv_d = dramp.tile([NCORES, 1, 64], bf16, name="recv_d")
        nc.sync.dma_start(send_d[:, 0, :], dummy_sb[:, :])
        nc.gpsimd.collective_compute(
            "AllToAll", mybir.AluOpType.bypass,
            replica_groups=[list(range(NCORES))],
            ins=[send_d.opt()], outs=[recv_d.opt()])

        # ---- load inputs ----
        xT_sb = [bigp.tile([P, N], bf16, name=f"xT_sb_{cb}") for cb in range(CB)]
        wqkT_sb = bigp.tile([P, CB * 384], bf16, name="wqkT_sb")
        wvT_sb = bigp.tile([P, CB * 192], bf16, name="wvT_sb")
        woT_sb = bigp.tile([P, CB * C], bf16, name="woT_sb")
        bo_sb = bigp.tile([P, CB], f32, name="bo_sb")
        ones_sb = constp.tile([P, 1], f32, name="ones_sb")
        nc.vector.memset(ones_sb[:, :], 1.0)
        warm_sb = constp.tile([P, 1], f32, name="warm_sb")
        nc.scalar.activation(warm_sb[0:1, :], ones_sb[0:1, 0:1], Exp, scale=SCALE)
        # weights on the ACT DMA ring, x on the sync ring: parallel transfers
        for cb in range(CB):
            nc.scalar.dma_start(wqkT_sb[:, cb * 384:(cb + 1) * 384], wqkT[cb * P:(cb + 1) * P, :])
        for cb in range(CB):
            nc.sync.dma_start(xT_sb[cb][:, :], xT[cb * P:(cb + 1) * P, :])
            nc.scalar.dma_start(wvT_sb[:, cb * 192:(cb + 1) * 192], wvT[cb * P:(cb + 1) * P, :])
        for cb in range(CB):
            nc.scalar.dma_start(woT_sb[:, cb * C:(cb + 1) * C], woT[cb * P:(cb + 1) * P, :])
            nc.scalar.dma_start(bo_sb[:, cb:cb + 1], bo_d[cb * P:(cb + 1) * P, :])

        # ---- PE warmup during loads: un-throttle HAM before projections ----
        wtile = constp.tile([P, 640], bf16, name="wtile")
        nc.vector.memset(wtile[:, :], 0.001)
        for w in range(24):
            wps = psS.tile([P, QCH], f32, name=f"wps_{w}", tag="psS")
            nc.tensor.matmul(wps[:, 0:QCH], lhsT=wtile[:, 0:128],
                             rhs=wtile[:, 128:640], start=True, stop=True)

        # ---- Q/K projections: per head j, psum = [q_j (rows 0:64) | k_j (64:128)]
        qT_sb = bigp.tile([P, HPC * N], bf16, name="qT_sb")
        kT_sb = bigp.tile([P, HPC * N], bf16, name="kT_sb")
        for j in range(HPC):
            for qn in range(4):
                ps = psS.tile([P, QCH], f32, name=f"pj_{j}_{qn}", tag="psS")
                for cb in range(CB):
                    nc.tensor.matmul(
                        ps[:, 0:QCH],
                        lhsT=wqkT_sb[:, cb * 384 + j * 128: cb * 384 + (j + 1) * 128],
                        rhs=xT_sb[cb][:, qn * QCH: (qn + 1) * QCH],
                        start=(cb == 0), stop=(cb == CB - 1),
                    )
                cols = slice(j * N + qn * QCH, j * N + (qn + 1) * QCH)
                nc.vector.tensor_copy(qT_sb[0:64, cols], ps[0:64, 0:QCH])
                nc.scalar.copy(kT_sb[64:128, cols], ps[64:128, 0:QCH])
            # duplicate head dims into the other partition half (row-tile pairing)
            blk = slice(j * N, (j + 1) * N)
            nc.sync.dma_start(qT_sb[64:128, blk], qT_sb[0:64, blk])
            nc.sync.dma_start(kT_sb[0:64, blk], kT_sb[64:128, blk])

        # ---- V projection (all key blocks upfront) ----
        v_sb = bigp.tile([P, KB * VW], bf16, name="v_sb")
        for nb in range(KB):
            ps = psS.tile([P, QCH], f32, name=f"vps_{nb}", tag="psS")
            for cb in range(CB):
                nc.tensor.matmul(
                    ps[:, 0:192],
                    lhsT=xT_sb[cb][:, nb * P:(nb + 1) * P],
                    rhs=wvT_sb[:, cb * 192:(cb + 1) * 192],
                    start=(cb == 0), stop=(cb == CB - 1),
                )
            vv = v_sb[:, nb * VW:(nb + 1) * VW].rearrange("p (h w) -> p h w", h=HPC)
            pp = ps[:, 0:192].rearrange("p (h w) -> p h w", h=HPC)
            nc.vector.tensor_copy(vv[:, :, 0:64], pp[:, :, :])
            nc.vector.memset(vv[:, :, 64:65], 1.0)

        # ---- attention, query-half-major ----
        # ctxT[ph]: [64, HPC*QP] normalized context for this phase
        ctxT_sb = [bigp.tile([64, HPC * QP], bf16, name=f"ctxT_sb_{ph}")
                   for ph in range(2)]
        # ctxTf[jp]: [128, u(2) x ph(2) x b(2) x RQ] assembled for outproj
        ctxTf_sb = [bigp.tile([P, 2 * 2 * 2 * RQ], bf16, name=f"ctxTf_sb_{jp}")
                    for jp in range(HPC)]
        db_t = [normp.tile([64, QP], f32, name=f"db_{i}") for i in range(2)]
        rb_t = [normp.tile([64, QP], f32, name=f"rb_{i}") for i in range(2)]

        def score_exp(j, ph, kb, es_list):
            kcol = slice(j * N + kb * P, j * N + (kb + 1) * P)
            es_pair = []
            for half in range(2):
                qc = 2 * ph + half
                ro = slice(half * 64, (half + 1) * 64)
                sps = psS.tile([P, QCH], f32, name=f"sps_{j}_{ph}_{kb}_{half}", tag="psS")
                nc.tensor.matmul(
                    sps[:, :],
                    lhsT=kT_sb[ro, kcol],
                    rhs=qT_sb[ro, j * N + qc * QCH: j * N + (qc + 1) * QCH],
                    start=True, stop=True,
                )
                if (kb + half) % 2 == 0:
                    es = esp.tile([P, QCH], bf16, name=f"esA_{j}_{ph}_{kb}_{half}", tag="es")
                    nc.scalar.activation(es, sps, Exp, scale=SCALE)
                    es_pair.append(es)
                else:
                    es16 = esp.tile([P, QCH], i16, name=f"esD_{j}_{ph}_{kb}_{half}", tag="es")
                    nc.vector.tensor_scalar(
                        out=es16[:, :], in0=sps[:, :],
                        scalar1=SEXP_A, scalar2=SEXP_B,
                        op0=mybir.AluOpType.mult, op1=mybir.AluOpType.add)
                    es_pair.append(es16.bitcast(bf16))
            es_list.append(es_pair)

        def pv_step(j, cps, kb, es_list):
            for half in range(2):
                nc.tensor.matmul(
                    cps[0:65, half * QCH:(half + 1) * QCH],
                    lhsT=v_sb[:, kb * VW + j * 65: kb * VW + (j + 1) * 65],
                    rhs=es_list[kb][half][:, :],
                    start=(kb == 0), stop=(kb == KB - 1))

        def norm_send(j, ph, cps, send_h):
            # denominator row -> DRAM round trip -> partition broadcast
            cden = smallp.tile([65, QP], f32, name=f"cden_{j}_{ph}", tag="rec")
            nc.scalar.copy(cden[64:65, :], cps[64:65, :])
            rtmp = dramp.tile([1, QP], f32, name=f"rtmp_{j}_{ph}")
            nc.sync.dma_start(rtmp[:, :], cden[64:65, :])
            db = db_t[j % 2]
            rb = rb_t[j % 2]
            nc.sync.dma_start(db[0:64, :], rtmp[0:1, :].partition_broadcast(64))
            nc.vector.reciprocal_approx_fast(out=rb[0:64, :], in_=db[0:64, :])
            nc.vector.tensor_mul(
                ctxT_sb[ph][0:64, j * QP:(j + 1) * QP],
                cps[0:64, :], rb[0:64, :])
            for s in range(NCORES):
                nc.sync.dma_start(
                    send_h[s, j * 64:(j + 1) * 64, :],
                    ctxT_sb[ph][0:64, j * QP + s * RQ: j * QP + (s + 1) * RQ])

        def a2a(ph, send_h):
            recv_h = dramp.tile([NCORES, HPC * 64, RQ], bf16, name=f"recv_{ph}")
            nc.gpsimd.collective_compute(
                "AllToAll", mybir.AluOpType.bypass,
                replica_groups=[list(range(NCORES))],
                ins=[send_h.opt()], outs=[recv_h.opt()])
            return recv_h

        def scatter(ph, recv_h):
            # issued on the (otherwise idle) GPSIMD queue: these wait on the
            # collective, and on the shared sync queue that head-of-line wait
            # would block every later DMA in the kernel
            for s in range(NCORES):
                u, ro = divmod((s % 4) * 64, P)
                co = u * 4 * RQ + ph * 2 * RQ + (s // 4) * RQ
                for jp in range(HPC):
                    nc.gpsimd.dma_start(
                        ctxTf_sb[jp][ro:ro + 64, co: co + RQ],
                        recv_h[s, jp * 64:(jp + 1) * 64, :])

        recvs = []
        for ph in range(2):
            send_h = dramp.tile([NCORES, HPC * 64, RQ], bf16, name=f"send_{ph}")
            for j in range(HPC):
                es_list = []
                cps = psC.tile([65, QP], f32, name=f"cps_{j}_{ph}", tag="psC")
                for kb in range(KB):
                    score_exp(j, ph, kb, es_list)
                    if kb >= PVLAG:
                        pv_step(j, cps, kb - PVLAG, es_list)
                for kb in range(KB - PVLAG, KB):
                    pv_step(j, cps, kb, es_list)
                norm_send(j, ph, cps, send_h)
            recvs.append(a2a(ph, send_h))
            scatter(ph, recvs[ph])

        # ---- tail: output projection per phase, cbo-outer with psum
        # ping-pong so bias+store pipelines behind the next cbo's matmuls
        for ph in range(2):
            for cbo in range(CB):
                ops = psS.tile([P, 2 * RQ], f32, name=f"ops_{ph}_{cbo}", tag="psS")
                for jc in range(CB):
                    jp, u = divmod(jc, 2)
                    nc.tensor.matmul(
                        ops[:, :],
                        lhsT=woT_sb[:, jc * C + cbo * P: jc * C + (cbo + 1) * P],
                        rhs=ctxTf_sb[jp][:, u * 4 * RQ + ph * 2 * RQ:
                                         u * 4 * RQ + (ph + 1) * 2 * RQ],
                        start=(jc == 0), stop=(jc == CB - 1),
                    )
                osb = outp.tile([P, 2 * RQ], f32, name=f"osb_{ph}_{cbo}", tag="osb")
                nc.scalar.activation(osb, ops, Identity, bias=bo_sb[:, cbo:cbo + 1])
                nc.sync.dma_start(
                    out_d[cbo * P:(cbo + 1) * P, ph * 2 * RQ:(ph + 1) * 2 * RQ], osb)

        if dbg is not None:
            nc.sync.dma_start(dbg["qT"][:, :], qT_sb[:, :])
            nc.sync.dma_start(dbg["kT"][:, :], kT_sb[:, :])
            nc.sync.dma_start(dbg["v"][:, :], v_sb[:, :])
            for ph in range(2):
                nc.sync.dma_start(dbg["ctxT"][:, ph * HPC * QP:(ph + 1) * HPC * QP],
                                  ctxT_sb[ph][:, :])
            for jp in range(HPC):
                nc.sync.dma_start(dbg["ctxTf"][:, jp * 1024:(jp + 1) * 1024],
                                  ctxTf_sb[jp][:, :])


def build(debug_outs=False):
    nc = bacc.Bacc("TRN2", target_bir_lowering=False, debug=False, num_devices=NCORES)
    xT = nc.dram_tensor("xT", [C, N], bf16, kind="ExternalInput").ap()
    wqkT = nc.dram_tensor("wqkT", [C, HPC * 128], bf16, kind="ExternalInput").ap()
    wvT = nc.dram_tensor("wvT", [C, HPC * HD], bf16, kind="ExternalInput").ap()
    woT = nc.dram_tensor("woT", [C, C], bf16, kind="ExternalInput").ap()
    bo_d = nc.dram_tensor("bo", [C, 1], f32, kind="ExternalInput").ap()
    out_d = nc.dram_tensor("out", [C, 4 * RQ], f32, kind="ExternalOutput").ap()
    dbg = None
    if debug_outs:
        dbg = {
            "qT": nc.dram_tensor("dbg_qT", [P, HPC * N], bf16, kind="ExternalOutput").ap(),
            "kT": nc.dram_tensor("dbg_kT", [P, HPC * N], bf16, kind="ExternalOutput").ap(),
            "v": nc.dram_tensor("dbg_v", [P, KB * VW], bf16, kind="ExternalOutput").ap(),
            "ctxT": nc.dram_tensor("dbg_ctxT", [64, 2 * HPC * QP], bf16, kind="ExternalOutput").ap(),
            "ctxTf": nc.dram_tensor("dbg_ctxTf", [P, HPC * 1024], bf16, kind="ExternalOutput").ap(),
        }
    with tile.TileContext(nc) as tc:
        _body(nc, tc, xT, wqkT, wvT, woT, bo_d, out_d, dbg)
    nc.compile()
    return nc


_NC = None


def _get_nc():
    global _NC
    if _NC is None:
        _NC = build()
    return _NC


def make_in_maps(x, Wq, Wk, Wv, Wo, bo):
    x = np.asarray(x, np.float32)
    woT = np.ascontiguousarray(np.asarray(Wo, np.float32).T).astype(ml_dtypes.bfloat16)
    bo_col = np.ascontiguousarray(np.asarray(bo, np.float32).reshape(C, 1))
    Wq = np.asarray(Wq, np.float32)
    Wk = np.asarray(Wk, np.float32)
    Wv = np.asarray(Wv, np.float32)
    in_maps = []
    for i in range(NCORES):
        b = i // 4
        r = i % 4
        heads = [r, r + 4, r + 8]
        # wqk columns per head block j: [q_hj (64) | k_hj (64)]
        blocks = []
        for h in heads:
            hs = slice(h * HD, (h + 1) * HD)
            blocks.append(Wq[hs])
            blocks.append(Wk[hs])
        wqk = np.concatenate(blocks, axis=0).T          # [768, 384]
        wv_rows = np.concatenate([Wv[h * HD:(h + 1) * HD] for h in heads], axis=0)
        in_maps.append({
            "xT": np.ascontiguousarray(x[b].T).astype(ml_dtypes.bfloat16),
            "wqkT": np.ascontiguousarray(wqk).astype(ml_dtypes.bfloat16),
            "wvT": np.ascontiguousarray(wv_rows.T).astype(ml_dtypes.bfloat16),
            "woT": woT,
            "bo": bo_col,
        })
    return in_maps


def unshard(results):
    out = np.empty((B, N, C), np.float32)
    for i, r in enumerate(results):
        o = r["out"]  # [768, 512]: cols [phA b0 | phA b1 | phB b0 | phB b1]
        out[0, i * RQ:(i + 1) * RQ, :] = o[:, 0 * RQ:1 * RQ].T
        out[1, i * RQ:(i + 1) * RQ, :] = o[:, 1 * RQ:2 * RQ].T
        out[0, QP + i * RQ:QP + (i + 1) * RQ, :] = o[:, 2 * RQ:3 * RQ].T
        out[1, QP + i * RQ:QP + (i + 1) * RQ, :] = o[:, 3 * RQ:4 * RQ].T
    return out


def kernel(x, Wq, Wk, Wv, Wo, bo):
    nc = _get_nc()
    in_maps = make_in_maps(x, Wq, Wk, Wv, Wo, bo)
    res = run_bass_kernel_spmd(nc, in_maps, core_ids=list(range(NCORES)))
    return unshard(res.results)
